# revision 4
# baseline (speedup 1.0000x reference)
"""GAT forward on 8 Trainium2 NeuronCores — one attention head per core.

Math (per head, all [4096] nodes):
    h   = x @ W                      [N, 128]
    ci  = h @ w_i  (per-node)        [N]
    cj  = h @ w_j  (per-node)        [N]
    e^T[j, i] = exp(leaky_relu(ci[i] + cj[j] + M[j, i]))   (M = 0 / -1e9 additive mask,
                M[j, i] = -1e9 where graph[j, i] == 0; masked entries exp to exactly 0)
    yT[f, i] = sum_j h[j, f] * eT[j, i]        (PE matmul, e as moving operand)
    rs[i]    = sum_j eT[j, i]                  (PE matmul vs ones column)
    y[i, f]  = yT[f, i] / rs[i] + (x @ W_r_head)[i, f]     (+ bias on host)

Layout notes:
  - Scores are computed TRANSPOSED (j on partitions) so the adjacency mask loads in
    natural row order and e feeds the PE as the moving operand with no on-chip
    transpose of the [N, N] matrix.
  - i is split in two 2048-wide halves so PSUM holds yT-half (4 banks) + rowsum
    (4 banks) simultaneously.
  - Elementwise softmax numerator is 3 passes: DVE scalar_tensor_tensor
    (ciB + cj[j]) + M, ACT Prelu(alpha=0.2), ACT Exp -> float32r.
  - Matmuls against the big e matrix run as float32r (full PE rate); the
    x-projections (h, resid) run in exact fp32.
"""
import sys

sys.path.insert(0, "/opt/trn_rl_repo")
from contextlib import ExitStack

import numpy as np
import ml_dtypes

import concourse.bass as bass
import concourse.tile as tile
from concourse import bacc, mybir
from concourse.bass_utils import run_bass_kernel_spmd

dt = mybir.dt
F32, F32R, BF16 = dt.float32, dt.float32r, dt.bfloat16
AF = mybir.ActivationFunctionType
OP = mybir.AluOpType

N = 4096
IN_F = 512
HF = 128
HEADS = 8
SLOPE = 0.2
MASK_NEG = -1.0e9
HALF = 2048
NJT = N // 128  # 32 j-tiles
NMC = IN_F // 128  # 4 contraction chunks over in-features

_prog = None


def build_program():
    nc = bacc.Bacc("TRN2", target_bir_lowering=False, debug=False)
    xT_d = nc.dram_tensor("xT", [IN_F, N], F32, kind="ExternalInput").ap()
    mask_d = nc.dram_tensor("mask", [N, N], BF16, kind="ExternalInput").ap()
    W_d = nc.dram_tensor("W", [IN_F, HF], F32, kind="ExternalInput").ap()
    wi_d = nc.dram_tensor("wi", [HF, 1], F32, kind="ExternalInput").ap()
    wj_d = nc.dram_tensor("wj", [HF, 1], F32, kind="ExternalInput").ap()
    Wr_d = nc.dram_tensor("Wr", [IN_F, HF], F32, kind="ExternalInput").ap()
    eye_d = nc.dram_tensor("eye", [128, 128], F32, kind="ExternalInput").ap()
    y_d = nc.dram_tensor("y", [N, HF], F32, kind="ExternalOutput").ap()

    with tile.TileContext(nc) as tc, ExitStack() as ctx:
        persist = ctx.enter_context(tc.tile_pool(name="persist", bufs=1))
        h_sb = persist.tile([128, N], F32R, tag="h")  # h[j,f], slice jt -> j-tile
        resid_sb = persist.tile([128, N], F32, tag="resid")  # resid[i,f] per i-tile
        ciB = persist.tile([128, N], F32, tag="ciB")  # ci broadcast along partitions
        cjT = persist.tile([128, NJT], F32, tag="cjT")  # cj[j] as per-partition cols
        eye_sb = persist.tile([128, 128], F32, tag="eye")
        ones_r = persist.tile([128, 1], F32R, tag="ones")

        nc.sync.dma_start(eye_sb[:], eye_d)
        ones_f = persist.tile([128, 1], F32, tag="ones_f")
        nc.vector.memset(ones_f[:], 1.0)
        nc.vector.tensor_copy(ones_r[:], ones_f[:])

        # ---------- Phase 1: projections ----------
        with ExitStack() as p1:
            ph1 = p1.enter_context(tc.tile_pool(name="ph1", bufs=1))
            ps_small = p1.enter_context(tc.tile_pool(name="ps1s", bufs=2, space="PSUM"))
            ps_wide = p1.enter_context(tc.tile_pool(name="ps1w", bufs=2, space="PSUM"))
            ps_cjp = p1.enter_context(tc.tile_pool(name="ps1c", bufs=1, space="PSUM"))

            xT_sb = ph1.tile([128, NMC * N], F32, tag="xT")  # [m-chunk, j] chunks
            for mc in range(NMC):
                nc.sync.dma_start(
                    xT_sb[:, mc * N : (mc + 1) * N], xT_d[mc * 128 : (mc + 1) * 128, :]
                )
            W_sb = ph1.tile([128, NMC * HF], F32, tag="W")
            Wr_sb = ph1.tile([128, NMC * HF], F32, tag="Wr")
            for mc in range(NMC):
                nc.sync.dma_start(
                    W_sb[:, mc * HF : (mc + 1) * HF], W_d[mc * 128 : (mc + 1) * 128, :]
                )
                nc.sync.dma_start(
                    Wr_sb[:, mc * HF : (mc + 1) * HF],
                    Wr_d[mc * 128 : (mc + 1) * 128, :],
                )
            wi_sb = ph1.tile([128, 1], F32, tag="wi")
            nc.sync.dma_start(wi_sb[:], wi_d)
            wj_sb = ph1.tile([128, 1], F32, tag="wj")
            nc.sync.dma_start(wj_sb[:], wj_d)

            hT_sb = ph1.tile([128, N], F32, tag="hT")  # hT[f, j]

            # h[j, f] per j-tile (exact fp32): lhsT = xT chunk, rhs = W chunk
            for jt in range(NJT):
                ps = ps_small.tile([128, HF], F32, tag="ps_s")
                for mc in range(NMC):
                    nc.tensor.matmul(
                        ps[:],
                        xT_sb[:, mc * N + jt * 128 : mc * N + (jt + 1) * 128],
                        W_sb[:, mc * HF : (mc + 1) * HF],
                        start=(mc == 0),
                        stop=(mc == NMC - 1),
                    )
                nc.vector.tensor_copy(h_sb[:, jt * 128 : (jt + 1) * 128], ps[:])

            # resid[i, f] per i-tile (exact fp32): lhsT = xT chunk, rhs = Wr chunk
            for it in range(NJT):
                ps = ps_small.tile([128, HF], F32, tag="ps_s")
                for mc in range(NMC):
                    nc.tensor.matmul(
                        ps[:],
                        xT_sb[:, mc * N + it * 128 : mc * N + (it + 1) * 128],
                        Wr_sb[:, mc * HF : (mc + 1) * HF],
                        start=(mc == 0),
                        stop=(mc == NMC - 1),
                    )
                nc.vector.tensor_copy(resid_sb[:, it * 128 : (it + 1) * 128], ps[:])

            # hT[f, j]: lhsT = W chunk [m, f], rhs = xT chunk [m, j]
            for nck in range(N // 512):
                ps = ps_wide.tile([128, 512], F32, tag="ps_w")
                for mc in range(NMC):
                    nc.tensor.matmul(
                        ps[:],
                        W_sb[:, mc * HF : (mc + 1) * HF],
                        xT_sb[:, mc * N + nck * 512 : mc * N + (nck + 1) * 512],
                        start=(mc == 0),
                        stop=(mc == NMC - 1),
                    )
                nc.scalar.copy(hT_sb[:, nck * 512 : (nck + 1) * 512], ps[:])

            # ci row [1, N] -> broadcast; cj column-form [128, NJT]
            ci_row = ph1.tile([1, N], F32, tag="ci_row")
            for nck in range(N // 512):
                ps = ps_wide.tile([1, 512], F32, tag="ps_w")
                nc.tensor.matmul(
                    ps[:],
                    wi_sb[:],
                    hT_sb[:, nck * 512 : (nck + 1) * 512],
                    start=True,
                    stop=True,
                )
                nc.scalar.copy(ci_row[0:1, nck * 512 : (nck + 1) * 512], ps[:])
            nc.gpsimd.partition_broadcast(ciB[:], ci_row[0:1, :])

            ps_cj = ps_cjp.tile([128, NJT], F32, tag="ps_cj")
            for jt in range(NJT):
                nc.tensor.matmul(
                    ps_cj[:, jt : jt + 1],
                    hT_sb[:, jt * 128 : (jt + 1) * 128],
                    wj_sb[:],
                    start=(jt == 0),
                    stop=(jt == NJT - 1),
                )
            nc.vector.tensor_copy(cjT[:], ps_cj[:])

        # ---------- Phase 2: attention ----------
        ph2 = ctx.enter_context(tc.tile_pool(name="ph2", bufs=3))
        upool = ctx.enter_context(tc.tile_pool(name="upool", bufs=2))
        fin = ctx.enter_context(tc.tile_pool(name="fin", bufs=2))
        outp = ctx.enter_context(tc.tile_pool(name="outp", bufs=4))

        for half in range(2):
            i0 = half * HALF
            with ExitStack() as pmm_ctx:
                pmm = pmm_ctx.enter_context(
                    tc.tile_pool(name=f"pmm{half}", bufs=1, space="PSUM")
                )
                yT_ps = pmm.tile([128, HALF], F32, tag="yT")
                rs_ps = pmm.tile([1, HALF], F32, tag="rs")

                for jt in range(NJT):
                    m_t = ph2.tile([128, HALF], BF16, tag="m")
                    nc.sync.dma_start(
                        m_t[:], mask_d[jt * 128 : (jt + 1) * 128, i0 : i0 + HALF]
                    )
                    IN = ph2.tile([128, HALF], F32, tag="IN")
                    nc.vector.scalar_tensor_tensor(
                        IN[:],
                        ciB[:, i0 : i0 + HALF],
                        cjT[:, jt : jt + 1],
                        m_t[:],
                        op0=OP.add,
                        op1=OP.add,
                    )
                    u = upool.tile([128, HALF], F32, tag="u")
                    nc.scalar.activation(u[:], IN[:], AF.Prelu, alpha=SLOPE)
                    e_r = ph2.tile([128, HALF], F32R, tag="e")
                    nc.scalar.activation(e_r[:], u[:], AF.Exp)

                    hr = h_sb[:, jt * 128 : (jt + 1) * 128]
                    for c in range(HALF // 512):
                        nc.tensor.matmul(
                            yT_ps[:, c * 512 : (c + 1) * 512],
                            hr,
                            e_r[:, c * 512 : (c + 1) * 512],
                            start=(jt == 0),
                            stop=(jt == NJT - 1),
                        )
                    for c in range(HALF // 512):
                        nc.tensor.matmul(
                            rs_ps[0:1, c * 512 : (c + 1) * 512],
                            ones_r[:],
                            e_r[:, c * 512 : (c + 1) * 512],
                            start=(jt == 0),
                            stop=(jt == NJT - 1),
                        )

                yT_sb = fin.tile([128, HALF], F32, tag="yT_sb")
                nc.vector.tensor_copy(yT_sb[:], yT_ps[:])
                rs_sb = fin.tile([1, HALF], F32, tag="rs_sb")
                nc.scalar.copy(rs_sb[:], rs_ps[:])

            with ExitStack() as pf_ctx:
                pfin = pf_ctx.enter_context(
                    tc.tile_pool(name=f"pfin{half}", bufs=1, space="PSUM")
                )
                # rowsum -> column form [128, 16] via PE transpose, then reciprocal
                rsT_ps = pfin.tile([128, HALF // 128], F32, tag="rsT")
                for c in range(HALF // 128):
                    nc.tensor.transpose(
                        rsT_ps[:, c : c + 1],
                        rs_sb[0:1, c * 128 : (c + 1) * 128],
                        eye_sb[0:1, 0:1],
                    )
                rsT_sb = fin.tile([128, HALF // 128], F32, tag="rsT_sb")
                nc.vector.tensor_copy(rsT_sb[:], rsT_ps[:])
                recipT = fin.tile([128, HALF // 128], F32, tag="recipT")
                nc.vector.reciprocal(recipT[:], rsT_sb[:])

                tr_ps = pfin.tile([128, HALF], F32, tag="tr")
                for gi in range(HALF // 128):
                    nc.tensor.transpose(
                        tr_ps[:, gi * 128 : (gi + 1) * 128],
                        yT_sb[:, gi * 128 : (gi + 1) * 128],
                        eye_sb[:],
                    )
                for gi in range(HALF // 128):
                    g = half * (HALF // 128) + gi
                    ob = outp.tile([128, HF], F32, tag="ob")
                    nc.vector.scalar_tensor_tensor(
                        ob[:],
                        tr_ps[:, gi * 128 : (gi + 1) * 128],
                        recipT[:, gi : gi + 1],
                        resid_sb[:, g * 128 : (g + 1) * 128],
                        op0=OP.mult,
                        op1=OP.add,
                    )
                    nc.sync.dma_start(y_d[g * 128 : (g + 1) * 128, :], ob[:])

    nc.compile()
    return nc


def _get_program():
    global _prog
    if _prog is None:
        _prog = build_program()
    return _prog


def _prepare_in_maps(x, graph, W, w_i, w_j, W_r):
    xT = np.ascontiguousarray(x.T).astype(np.float32, copy=False)
    mask = np.where(graph > 0, np.float32(0.0), np.float32(MASK_NEG)).astype(
        ml_dtypes.bfloat16
    )
    eye = np.eye(128, dtype=np.float32)
    in_maps = []
    for c in range(HEADS):
        in_maps.append(
            {
                "xT": xT,
                "mask": mask,
                "W": np.ascontiguousarray(W[c]).astype(np.float32, copy=False),
                "wi": np.ascontiguousarray(w_i[c]).astype(np.float32, copy=False),
                "wj": np.ascontiguousarray(w_j[c]).astype(np.float32, copy=False),
                "Wr": np.ascontiguousarray(W_r[:, c * HF : (c + 1) * HF]).astype(
                    np.float32, copy=False
                ),
                "eye": eye,
            }
        )
    return in_maps


def run(inputs, trace=False, **kwargs):
    """Run the SPMD kernel; returns (y_full, BassKernelResults)."""
    x = np.asarray(inputs["x"], dtype=np.float32)
    graph = np.asarray(inputs["graph"])
    W = np.asarray(inputs["W"], dtype=np.float32)
    w_i = np.asarray(inputs["w_i"], dtype=np.float32)
    w_j = np.asarray(inputs["w_j"], dtype=np.float32)
    W_r = np.asarray(inputs["W_r"], dtype=np.float32)
    bias = np.asarray(inputs["bias"], dtype=np.float32)

    nc = _get_program()
    in_maps = _prepare_in_maps(x, graph, W, w_i, w_j, W_r)
    br = run_bass_kernel_spmd(
        nc, in_maps, core_ids=list(range(HEADS)), trace=trace, **kwargs
    )
    y = np.concatenate([br.results[c]["y"] for c in range(HEADS)], axis=1)
    y = y + bias[None, :]
    return y.astype(np.float32), br


def kernel(**inputs):
    y, _ = run(inputs)
    return y
